# revision 41
# baseline (speedup 1.0000x reference)
"""Multi-head attention (dense_transformer) on 8 TRN2 NeuronCores.

Reference computation (B=1, N=4096, D=512, 8 heads, head_dim 64):
    q = x @ Wq.T ; k, v = split(x @ Wkv.T)
    attn = softmax_masked(q k^T * scale)   # diagonal masked to zero
    out = (attn @ v) @ Wproj.T + bproj

Sharding: head-parallel — core h computes head h end to end, including its
partial output projection out_h = (attn_h @ v_h) @ Wproj[:, 64h:64h+64].T.
The host sums the 8 partials and adds the bias. No cross-core comm.

Per-core layout is fully "transposed" (channels on partitions) so that the
softmax normalizer comes out of the PV matmul via an appended ones column:
    XT  [c=512, n=4096]  (TensorE transposes of x tiles)
    QT/KT [128, 4096]    rows 0..63 = head projection, rows 64..127 = copy
                         (duplicate halves enable 2-way row-packed QK matmuls)
    ST strip j = K_j @ Q^T -> [128 keys, 512 queries] in PSUM
    E = exp(scale * ST)  (ScalarE, no max subtraction needed: |scores| < ~2)
    diagonal blocks of E multiplied by (1 - I) to zero self-attention
    OT'[65, 512] += V'_j^T @ E_j  over key strips; row 64 = sum of exps
    PO[tok, 512] = OT'[0:64, tok-block].T @ WprojSlice^T
    out[tok, :]  = PO * (1/sums)[tok]   (per-partition tensor_scalar)

The preamble (XT transposes, KT/V' strips, QT chunks) is produced
just-in-time inside chunk 0 so the ScalarEngine starts exp'ing early.
"""

import numpy as np

import concourse.bass as bass
import concourse.tile as tile
from concourse import bacc, mybir
from concourse.bass_utils import run_bass_kernel_spmd

F32 = mybir.dt.float32
F32R = mybir.dt.float32r
BF16 = mybir.dt.bfloat16
EXP = mybir.ActivationFunctionType.Exp

N = 4096
D = 512
NH = 8
HD = 64
NQC = 8          # query chunks of 512
QC = 512
NST = 32         # key strips of 128
GS = 3           # key strips per exp group

LAST_EXEC_TIME_NS = None
_BUILD_CACHE = {}


def _groups():
    """Key-strip groups per query chunk: [(start, size), ...]."""
    out = []
    j = 0
    while j < NST:
        out.append((j, min(GS, NST - j)))
        j += GS
    return out


def _build(scale_val: float):
    nc = bacc.Bacc("TRN2", target_bir_lowering=False, debug=False)

    x_d = nc.dram_tensor("xinp", [N, D], BF16, kind="ExternalInput").ap()
    wq_d = nc.dram_tensor("wq", [128, 4, 128], BF16, kind="ExternalInput").ap()
    wk_d = nc.dram_tensor("wk", [128, 4, 128], BF16, kind="ExternalInput").ap()
    wv_d = nc.dram_tensor("wv", [128, 4, HD + 2], BF16, kind="ExternalInput").ap()
    identb_d = nc.dram_tensor("identb", [128, 128], BF16, kind="ExternalInput").ap()
    mask_d = nc.dram_tensor("mask", [128, 128], BF16, kind="ExternalInput").ap()
    ones_d = nc.dram_tensor("onesw", [128, HD], BF16, kind="ExternalInput").ap()
    wp_d = nc.dram_tensor("wp", [HD, D], F32, kind="ExternalInput").ap()
    ident_d = nc.dram_tensor("ident", [1, 1], F32, kind="ExternalInput").ap()
    out_d = nc.dram_tensor("out", [N, D], F32, kind="ExternalOutput").ap()

    groups = _groups()
    ng = len(groups)

    with tile.TileContext(nc) as tc:
        with (
            tc.tile_pool(name="consts", bufs=1) as consts,
            tc.tile_pool(name="persist", bufs=1) as persist,
            tc.tile_pool(name="xin", bufs=8) as xin,
            tc.tile_pool(name="epool", bufs=4) as epool,
            tc.tile_pool(name="small", bufs=2) as small,
            tc.tile_pool(name="outp", bufs=3) as outp,
            tc.tile_pool(name="ps_st", bufs=2, space="PSUM") as ps_st,
            tc.tile_pool(name="ps_ot", bufs=1, space="PSUM") as ps_ot,
            tc.tile_pool(name="ps_misc", bufs=1, space="PSUM") as ps_misc,
        ):
            # ---- constants ----
            wq_sb = consts.tile([128, 4, 128], BF16, tag="wq")
            wk_sb = consts.tile([128, 4, 128], BF16, tag="wk")
            wv_sb = consts.tile([128, 4, HD + 2], BF16, tag="wv")
            identb_sb = consts.tile([128, 128], BF16, tag="identb")
            mask_sb = consts.tile([128, 128], BF16, tag="mask")
            ones_sb = consts.tile([128, HD], BF16, tag="ones")
            wp_f32 = consts.tile([HD, D], F32, tag="wpf")
            wp_sb = consts.tile([HD, D], F32R, tag="wp")
            ident_sb = consts.tile([1, 1], F32, tag="ident")
            nc.sync.dma_start(out=identb_sb, in_=identb_d)
            x_pre = []
            for t in range(8):
                x_t = xin.tile([128, D], BF16, tag="xin", name=f"x{t}")
                nc.sync.dma_start(out=x_t, in_=x_d[t * 128:(t + 1) * 128, :])
                x_pre.append(x_t)
            nc.sync.dma_start(out=wq_sb, in_=wq_d)
            nc.sync.dma_start(out=wk_sb, in_=wk_d)
            nc.sync.dma_start(out=wv_sb, in_=wv_d)
            nc.sync.dma_start(out=mask_sb, in_=mask_d)
            nc.sync.dma_start(out=ones_sb, in_=ones_d)
            nc.sync.dma_start(out=wp_f32, in_=wp_d)
            nc.vector.tensor_copy(wp_sb, wp_f32)
            nc.sync.dma_start(out=ident_sb, in_=ident_d)

            # preload the exp activation table while DMAs stream
            scratch = consts.tile([1, 2], F32, tag="scratch")
            nc.vector.memset(scratch, 0.0)
            nc.scalar.activation(scratch, scratch, EXP)

            # ---- persistent tensors ----
            xt = persist.tile([128, 4, N], BF16, tag="xt")     # XT[c%128, c//128, n]
            qt = persist.tile([128, N], BF16, tag="qt")        # QT duplicated halves
            kt = persist.tile([128, N], BF16, tag="kt")
            vp = persist.tile([128, NST, HD + 2], BF16, tag="vp")

            # ---- preamble: XT via TensorE transposes, produced lazily ----
            t_done = [False] * NST
            kt_done = [False] * NQC
            v_done = [False] * NST
            qt_done = [False] * NQC

            # tokens 0..1023 go through the TensorEngine (their DMAs are
            # issued first so the xbar bulk transfer doesn't delay them);
            # tokens 1024..4095 arrive via xbar DMA transpose behind them.
            for b0 in range(1024, N, 1024):
                for cb in range(4):
                    nc.sync.dma_start_transpose(
                        out=xt[:, cb, b0:b0 + 1024],
                        in_=x_d[b0:b0 + 1024, cb * 128:(cb + 1) * 128],
                    )
            for t_ in range(8, NST):
                t_done[t_] = True

            def prod_t(t):
                if t_done[t]:
                    return
                t_done[t] = True
                x_t = x_pre[t]
                tr = ps_st.tile([128, D], BF16, tag="st", name=f"tr{t}")
                for cb in range(4):
                    nc.tensor.transpose(
                        tr[:, cb * 128:(cb + 1) * 128],
                        x_t[:, cb * 128:(cb + 1) * 128],
                        identb_sb,
                    )
                nc.vector.tensor_copy(
                    xt[:, 0:4, t * 128:(t + 1) * 128],
                    tr.rearrange("p (cb tt) -> p cb tt", cb=4),
                )

            def prod_xt(qc):
                for t in range(4 * qc, 4 * qc + 4):
                    prod_t(t)

            def prod_kt(qc):
                if kt_done[qc]:
                    return
                kt_done[qc] = True
                prod_xt(qc)
                sl = slice(qc * QC, (qc + 1) * QC)
                pp = ps_st.tile([128, QC], F32, tag="st")
                for cc in range(4):
                    nc.tensor.matmul(
                        pp, wk_sb[:, cc, :], xt[:, cc, sl],
                        start=(cc == 0), stop=(cc == 3),
                    )
                nc.vector.tensor_copy(kt[:, sl], pp)

            def prod_qt(qc):
                if qt_done[qc]:
                    return
                qt_done[qc] = True
                prod_xt(qc)
                sl = slice(qc * QC, (qc + 1) * QC)
                pp = ps_misc.tile([128, QC], F32, tag="misc", name=f"qtp{qc}")
                for cc in range(4):
                    nc.tensor.matmul(
                        pp, wq_sb[:, cc, :], xt[:, cc, sl],
                        start=(cc == 0), stop=(cc == 3),
                    )
                nc.vector.tensor_copy(qt[:, sl], pp)

            def prod_v(t):
                if v_done[t]:
                    return
                v_done[t] = True
                prod_xt(t // 4)
                vv = ps_st.tile([128, HD + 2], F32, tag="st")
                for cc in range(4):
                    nc.tensor.matmul(
                        vv, xt[:, cc, t * 128:(t + 1) * 128], wv_sb[:, cc, :],
                        start=(cc == 0), stop=(cc == 3),
                    )
                nc.vector.tensor_copy(vp[:, t, :], vv)
                nc.vector.tensor_copy(vp[:, t, HD:HD + 1], ones_sb[:, 0:1])

            def prod_for_group(c, g):
                if c > 0 or g >= ng:
                    return
                j0, gsz = groups[g]
                for i in range(gsz):
                    j = j0 + i
                    prod_kt(j // 4)
                    prod_v(j)

            # ---- per-chunk state ----
            ot_tiles = {}
            e_tiles = {}
            st_tiles = {}
            sums_tiles = {}
            ots_tiles = {}

            def emit_qk(c, g):
                j0, gsz = groups[g]
                qsl = slice(c * QC, (c + 1) * QC)
                st = ps_st.tile([128, GS, QC], F32, tag="st")
                st_tiles[(c, g)] = st
                for i in range(gsz):
                    j = j0 + i
                    ro = 64 * (j % 2)
                    nc.tensor.matmul(
                        st[:, i, :],
                        kt[ro:ro + 64, j * 128:(j + 1) * 128],
                        qt[ro:ro + 64, qsl],
                        start=True,
                        stop=True,
                    )

            def emit_exp(c, g):
                j0, gsz = groups[g]
                e_t = epool.tile([128, GS, QC], BF16, tag="e")
                e_tiles[(c, g)] = e_t
                nc.scalar.activation(
                    e_t[:, 0:gsz, :],
                    st_tiles.pop((c, g))[:, 0:gsz, :],
                    EXP,
                    scale=scale_val,
                )
                for i in range(gsz):
                    j = j0 + i
                    r = j - 4 * c
                    if 0 <= r < 4:
                        blk = e_t[:, i, r * 128:(r + 1) * 128]
                        nc.vector.tensor_mul(blk, blk, mask_sb)

            def emit_pv(c, g):
                j0, gsz = groups[g]
                if g == 0:
                    ot_tiles[c] = ps_ot.tile([128, QC], F32, tag="ot", name=f"ot{c}")
                ot = ot_tiles[c]
                e_t = e_tiles.pop((c, g))
                for i in range(gsz):
                    j = j0 + i
                    nc.tensor.matmul(
                        ot[0:HD + 1, :],
                        vp[:, j, 0:HD + 1],
                        e_t[:, i, :],
                        start=(j == 0),
                        stop=(j == NST - 1),
                        skip_group_check=True,
                    )

            def emit_copies(c):
                # pull sums row + unnormalized OT out of PSUM; frees the ot bank
                ot = ot_tiles.pop(c)
                sums_sb = small.tile([1, QC], F32, tag="sums")
                nc.vector.tensor_copy(sums_sb, ot[HD:HD + 1, :])
                ots_sb = small.tile([HD, QC], F32R, tag="ots")
                nc.vector.tensor_copy(ots_sb, ot[0:HD, :])
                sums_tiles[c] = sums_sb
                ots_tiles[c] = ots_sb

            def emit_norm_po(c):
                pool, ptag = ((ps_st, "st") if c == NQC - 1
                              else (ps_misc, "misc"))
                sums_sb = sums_tiles.pop(c)
                ots_sb = ots_tiles.pop(c)
                ts_ps = pool.tile([128, 4], F32, tag=ptag, name=f"ts{c}")
                for tb in range(4):
                    nc.tensor.transpose(
                        ts_ps[:, tb:tb + 1],
                        sums_sb[0:1, tb * 128:(tb + 1) * 128],
                        ident_sb[0:1, 0:1],
                    )
                recip_sb = small.tile([128, 4], F32, tag="recip")
                with nc.allow_low_precision(reason="fp32 reciprocal"):
                    nc.vector.reciprocal(recip_sb, ts_ps)
                for tb in range(4):
                    po = pool.tile([128, QC], F32, tag=ptag, name=f"po{c}_{tb}")
                    nc.tensor.matmul(
                        po,
                        ots_sb[:, tb * 128:(tb + 1) * 128],
                        wp_sb,
                        start=True,
                        stop=True,
                    )
                    o_sb = outp.tile([128, D], F32, tag="outs")
                    nc.vector.tensor_scalar_mul(o_sb, po, recip_sb[:, tb:tb + 1])
                    row = c * QC + tb * 128
                    nc.sync.dma_start(out=out_d[row:row + 128, :], in_=o_sb)

            # ---- flat software pipeline across all (chunk, group) steps ----
            # QK leads exp by one step; PV trails exp by one step; each
            # chunk's normalization + projection is emitted two steps after
            # its last group so it never blocks the next chunk's QK path.
            seq = [(c, g) for c in range(NQC) for g in range(ng)]
            prod_for_group(0, 0)
            prod_qt(0)
            for i, (c, g) in enumerate(seq):
                emit_qk(c, g)
                if i > 0:
                    pc, pg = seq[i - 1]
                    emit_pv(pc, pg)
                    if pg == ng - 1:
                        emit_copies(pc)
                emit_exp(c, g)
                prod_for_group(c, g + 1)
                prod_for_group(c, g + 2)
                if i > 1:
                    ppc, ppg = seq[i - 2]
                    if ppg == ng - 1:
                        emit_norm_po(ppc)
                if g == 4:
                    prod_qt(min(c + 1, NQC - 1))
            emit_pv(*seq[-1])
            emit_copies(NQC - 1)
            emit_norm_po(NQC - 1)

    nc.compile()
    return nc


def _prep_inputs(x, scale, Wq, Wkv, Wproj):
    """Per-core input maps (head h on core h)."""
    import ml_dtypes
    bf = ml_dtypes.bfloat16
    x2 = np.ascontiguousarray(x.reshape(N, D)).astype(bf)
    mask = (1.0 - np.eye(128)).astype(np.float32)
    identb = np.eye(128, dtype=np.float32)
    onesw = np.ones((128, HD), dtype=np.float32)
    ident = np.ones((1, 1), dtype=np.float32)
    in_maps = []
    for h in range(NH):
        wqh = Wq[h * HD:(h + 1) * HD, :]                  # [64, 512]
        wkh = Wkv[h * HD:(h + 1) * HD, :]
        wvh = Wkv[D + h * HD:D + (h + 1) * HD, :]
        # lhsT [c, m] with m duplicated halves -> [128, 4x128]
        def lhsT_dup(w):
            a = np.concatenate([w.T, w.T], axis=1)        # [512, 128]
            return np.ascontiguousarray(
                a.reshape(4, 128, 128).transpose(1, 0, 2))
        # V' rhs [c, 66] -> [128, 4, 66] (col 64 becomes the ones column)
        b = np.concatenate(
            [wvh.T, np.zeros((D, 2), dtype=np.float32)], axis=1)
        wv_host = np.ascontiguousarray(
            b.reshape(4, 128, HD + 2).transpose(1, 0, 2))
        wp_host = np.ascontiguousarray(
            Wproj[:, h * HD:(h + 1) * HD].T, dtype=np.float32)  # [64, 512]
        in_maps.append({
            "xinp": x2,
            "wq": np.ascontiguousarray(lhsT_dup(wqh)).astype(bf),
            "wk": np.ascontiguousarray(lhsT_dup(wkh)).astype(bf),
            "wv": np.ascontiguousarray(wv_host).astype(bf),
            "identb": identb.astype(bf),
            "mask": mask.astype(bf),
            "onesw": onesw.astype(bf),
            "wp": wp_host,
            "ident": ident,
        })
    return in_maps


def kernel(x, H, W, scale, Wq, Wkv, Wproj, bproj, _trace=False):
    global LAST_EXEC_TIME_NS
    x = np.asarray(x, dtype=np.float32)
    Wq = np.asarray(Wq, dtype=np.float32)
    Wkv = np.asarray(Wkv, dtype=np.float32)
    Wproj = np.asarray(Wproj, dtype=np.float32)
    bproj = np.asarray(bproj, dtype=np.float32)
    scale_val = float(np.asarray(scale).reshape(-1)[0])

    key = round(scale_val, 12)
    nc = _BUILD_CACHE.get(key)
    if nc is None:
        nc = _build(scale_val)
        _BUILD_CACHE[key] = nc

    in_maps = _prep_inputs(x, scale, Wq, Wkv, Wproj)
    try:
        res = run_bass_kernel_spmd(
            nc, in_maps, core_ids=list(range(NH)), trace=_trace)
    except Exception:
        # transient NRT device errors recover on retry
        res = run_bass_kernel_spmd(
            nc, in_maps, core_ids=list(range(NH)), trace=_trace)
    LAST_EXEC_TIME_NS = res.exec_time_ns

    acc = np.zeros((N, D), dtype=np.float64)
    for h in range(NH):
        acc += res.results[h]["out"]
    out = (acc + bproj.astype(np.float64)).astype(np.float32)
    return out.reshape(1, N, D)


# revision 42
# speedup vs baseline: 1.0106x; 1.0106x over previous
"""Multi-head attention (dense_transformer) on 8 TRN2 NeuronCores.

Reference computation (B=1, N=4096, D=512, 8 heads, head_dim 64):
    q = x @ Wq.T ; k, v = split(x @ Wkv.T)
    attn = softmax_masked(q k^T * scale)   # diagonal masked to zero
    out = (attn @ v) @ Wproj.T + bproj

Sharding: head-parallel — core h computes head h end to end, including its
partial output projection out_h = (attn_h @ v_h) @ Wproj[:, 64h:64h+64].T.
The host sums the 8 partials and adds the bias. No cross-core comm.

Per-core layout is fully "transposed" (channels on partitions) so that the
softmax normalizer comes out of the PV matmul via an appended ones column:
    XT  [c=512, n=4096]  (TensorE transposes of x tiles)
    QT/KT [128, 4096]    rows 0..63 = head projection, rows 64..127 = copy
                         (duplicate halves enable 2-way row-packed QK matmuls)
    ST strip j = K_j @ Q^T -> [128 keys, 512 queries] in PSUM
    E = exp(scale * ST)  (ScalarE, no max subtraction needed: |scores| < ~2)
    diagonal blocks of E multiplied by (1 - I) to zero self-attention
    OT'[65, 512] += V'_j^T @ E_j  over key strips; row 64 = sum of exps
    PO[tok, 512] = OT'[0:64, tok-block].T @ WprojSlice^T
    out[tok, :]  = PO * (1/sums)[tok]   (per-partition tensor_scalar)

The preamble (XT transposes, KT/V' strips, QT chunks) is produced
just-in-time inside chunk 0 so the ScalarEngine starts exp'ing early.
"""

import numpy as np

import concourse.bass as bass
import concourse.tile as tile
from concourse import bacc, mybir
from concourse.bass_utils import run_bass_kernel_spmd

F32 = mybir.dt.float32
F32R = mybir.dt.float32r
BF16 = mybir.dt.bfloat16
EXP = mybir.ActivationFunctionType.Exp

N = 4096
D = 512
NH = 8
HD = 64
NQC = 8          # query chunks of 512
QC = 512
NST = 32         # key strips of 128
GS = 3           # key strips per exp group

LAST_EXEC_TIME_NS = None
_BUILD_CACHE = {}


def _groups():
    """Key-strip groups per query chunk: [(start, size), ...]."""
    out = []
    j = 0
    while j < NST:
        out.append((j, min(GS, NST - j)))
        j += GS
    return out


def _build(scale_val: float):
    nc = bacc.Bacc("TRN2", target_bir_lowering=False, debug=False)

    x_d = nc.dram_tensor("xinp", [N, D], BF16, kind="ExternalInput").ap()
    wq_d = nc.dram_tensor("wq", [128, 4, 128], BF16, kind="ExternalInput").ap()
    wk_d = nc.dram_tensor("wk", [128, 4, 128], BF16, kind="ExternalInput").ap()
    wv_d = nc.dram_tensor("wv", [128, 4, HD + 2], BF16, kind="ExternalInput").ap()
    identb_d = nc.dram_tensor("identb", [128, 128], BF16, kind="ExternalInput").ap()
    mask_d = nc.dram_tensor("mask", [128, 128], BF16, kind="ExternalInput").ap()
    ones_d = nc.dram_tensor("onesw", [128, HD], BF16, kind="ExternalInput").ap()
    wp_d = nc.dram_tensor("wp", [HD, D], F32, kind="ExternalInput").ap()
    ident_d = nc.dram_tensor("ident", [1, 1], F32, kind="ExternalInput").ap()
    out_d = nc.dram_tensor("out", [N, D], F32, kind="ExternalOutput").ap()

    groups = _groups()
    ng = len(groups)

    with tile.TileContext(nc) as tc:
        with (
            tc.tile_pool(name="consts", bufs=1) as consts,
            tc.tile_pool(name="persist", bufs=1) as persist,
            tc.tile_pool(name="xin", bufs=8) as xin,
            tc.tile_pool(name="epool", bufs=6) as epool,
            tc.tile_pool(name="small", bufs=2) as small,
            tc.tile_pool(name="outp", bufs=4) as outp,
            tc.tile_pool(name="ps_st", bufs=2, space="PSUM") as ps_st,
            tc.tile_pool(name="ps_ot", bufs=1, space="PSUM") as ps_ot,
            tc.tile_pool(name="ps_misc", bufs=1, space="PSUM") as ps_misc,
        ):
            # ---- constants ----
            wq_sb = consts.tile([128, 4, 128], BF16, tag="wq")
            wk_sb = consts.tile([128, 4, 128], BF16, tag="wk")
            wv_sb = consts.tile([128, 4, HD + 2], BF16, tag="wv")
            identb_sb = consts.tile([128, 128], BF16, tag="identb")
            mask_sb = consts.tile([128, 128], BF16, tag="mask")
            ones_sb = consts.tile([128, HD], BF16, tag="ones")
            wp_f32 = consts.tile([HD, D], F32, tag="wpf")
            wp_sb = consts.tile([HD, D], F32R, tag="wp")
            ident_sb = consts.tile([1, 1], F32, tag="ident")
            nc.sync.dma_start(out=identb_sb, in_=identb_d)
            x_pre = []
            for t in range(8):
                x_t = xin.tile([128, D], BF16, tag="xin", name=f"x{t}")
                nc.sync.dma_start(out=x_t, in_=x_d[t * 128:(t + 1) * 128, :])
                x_pre.append(x_t)
            nc.sync.dma_start(out=wq_sb, in_=wq_d)
            nc.sync.dma_start(out=wk_sb, in_=wk_d)
            nc.sync.dma_start(out=wv_sb, in_=wv_d)
            nc.sync.dma_start(out=mask_sb, in_=mask_d)
            nc.sync.dma_start(out=ones_sb, in_=ones_d)
            nc.sync.dma_start(out=wp_f32, in_=wp_d)
            nc.vector.tensor_copy(wp_sb, wp_f32)
            nc.sync.dma_start(out=ident_sb, in_=ident_d)

            # preload the exp activation table while DMAs stream
            scratch = consts.tile([1, 2], F32, tag="scratch")
            nc.vector.memset(scratch, 0.0)
            nc.scalar.activation(scratch, scratch, EXP)

            # ---- persistent tensors ----
            xt = persist.tile([128, 4, N], BF16, tag="xt")     # XT[c%128, c//128, n]
            qt = persist.tile([128, N], BF16, tag="qt")        # QT duplicated halves
            kt = persist.tile([128, N], BF16, tag="kt")
            vp = persist.tile([128, NST, HD + 2], BF16, tag="vp")

            # ---- preamble: XT via TensorE transposes, produced lazily ----
            t_done = [False] * NST
            kt_done = [False] * NQC
            v_done = [False] * NST
            qt_done = [False] * NQC

            # tokens 0..1023 go through the TensorEngine (their DMAs are
            # issued first so the xbar bulk transfer doesn't delay them);
            # tokens 1024..4095 arrive via xbar DMA transpose behind them.
            for b0 in range(1024, N, 1024):
                for cb in range(4):
                    nc.sync.dma_start_transpose(
                        out=xt[:, cb, b0:b0 + 1024],
                        in_=x_d[b0:b0 + 1024, cb * 128:(cb + 1) * 128],
                    )
            for t_ in range(8, NST):
                t_done[t_] = True

            def prod_t(t):
                if t_done[t]:
                    return
                t_done[t] = True
                x_t = x_pre[t]
                tr = ps_st.tile([128, D], BF16, tag="st", name=f"tr{t}")
                for cb in range(4):
                    nc.tensor.transpose(
                        tr[:, cb * 128:(cb + 1) * 128],
                        x_t[:, cb * 128:(cb + 1) * 128],
                        identb_sb,
                    )
                nc.vector.tensor_copy(
                    xt[:, 0:4, t * 128:(t + 1) * 128],
                    tr.rearrange("p (cb tt) -> p cb tt", cb=4),
                )

            def prod_xt(qc):
                for t in range(4 * qc, 4 * qc + 4):
                    prod_t(t)

            def prod_kt(qc):
                if kt_done[qc]:
                    return
                kt_done[qc] = True
                prod_xt(qc)
                sl = slice(qc * QC, (qc + 1) * QC)
                pp = ps_st.tile([128, QC], F32, tag="st")
                for cc in range(4):
                    nc.tensor.matmul(
                        pp, wk_sb[:, cc, :], xt[:, cc, sl],
                        start=(cc == 0), stop=(cc == 3),
                    )
                nc.vector.tensor_copy(kt[:, sl], pp)

            def prod_qt(qc):
                if qt_done[qc]:
                    return
                qt_done[qc] = True
                prod_xt(qc)
                sl = slice(qc * QC, (qc + 1) * QC)
                pp = ps_misc.tile([128, QC], F32, tag="misc", name=f"qtp{qc}")
                for cc in range(4):
                    nc.tensor.matmul(
                        pp, wq_sb[:, cc, :], xt[:, cc, sl],
                        start=(cc == 0), stop=(cc == 3),
                    )
                nc.vector.tensor_copy(qt[:, sl], pp)

            def prod_v(t):
                if v_done[t]:
                    return
                v_done[t] = True
                prod_xt(t // 4)
                vv = ps_st.tile([128, HD + 2], F32, tag="st")
                for cc in range(4):
                    nc.tensor.matmul(
                        vv, xt[:, cc, t * 128:(t + 1) * 128], wv_sb[:, cc, :],
                        start=(cc == 0), stop=(cc == 3),
                    )
                nc.vector.tensor_copy(vp[:, t, :], vv)
                nc.vector.tensor_copy(vp[:, t, HD:HD + 1], ones_sb[:, 0:1])

            def prod_for_group(c, g):
                if c > 0 or g >= ng:
                    return
                j0, gsz = groups[g]
                for i in range(gsz):
                    j = j0 + i
                    prod_kt(j // 4)
                    prod_v(j)

            # ---- per-chunk state ----
            ot_tiles = {}
            e_tiles = {}
            st_tiles = {}
            sums_tiles = {}
            ots_tiles = {}

            def emit_qk(c, g):
                j0, gsz = groups[g]
                qsl = slice(c * QC, (c + 1) * QC)
                st = ps_st.tile([128, GS, QC], F32, tag="st")
                st_tiles[(c, g)] = st
                for i in range(gsz):
                    j = j0 + i
                    ro = 64 * (j % 2)
                    nc.tensor.matmul(
                        st[:, i, :],
                        kt[ro:ro + 64, j * 128:(j + 1) * 128],
                        qt[ro:ro + 64, qsl],
                        start=True,
                        stop=True,
                    )

            def emit_exp(c, g):
                j0, gsz = groups[g]
                e_t = epool.tile([128, GS, QC], BF16, tag="e")
                e_tiles[(c, g)] = e_t
                nc.scalar.activation(
                    e_t[:, 0:gsz, :],
                    st_tiles.pop((c, g))[:, 0:gsz, :],
                    EXP,
                    scale=scale_val,
                )
                for i in range(gsz):
                    j = j0 + i
                    r = j - 4 * c
                    if 0 <= r < 4:
                        blk = e_t[:, i, r * 128:(r + 1) * 128]
                        nc.vector.tensor_mul(blk, blk, mask_sb)

            def emit_pv(c, g):
                j0, gsz = groups[g]
                if g == 0:
                    ot_tiles[c] = ps_ot.tile([128, QC], F32, tag="ot", name=f"ot{c}")
                ot = ot_tiles[c]
                e_t = e_tiles.pop((c, g))
                for i in range(gsz):
                    j = j0 + i
                    nc.tensor.matmul(
                        ot[0:HD + 1, :],
                        vp[:, j, 0:HD + 1],
                        e_t[:, i, :],
                        start=(j == 0),
                        stop=(j == NST - 1),
                        skip_group_check=True,
                    )

            def emit_copies(c):
                # pull sums row + unnormalized OT out of PSUM; frees the ot bank
                ot = ot_tiles.pop(c)
                sums_sb = small.tile([1, QC], F32, tag="sums")
                nc.vector.tensor_copy(sums_sb, ot[HD:HD + 1, :])
                ots_sb = small.tile([HD, QC], F32R, tag="ots")
                nc.vector.tensor_copy(ots_sb, ot[0:HD, :])
                sums_tiles[c] = sums_sb
                ots_tiles[c] = ots_sb

            def emit_norm_po(c):
                pool, ptag = ((ps_st, "st") if c == NQC - 1
                              else (ps_misc, "misc"))
                sums_sb = sums_tiles.pop(c)
                ots_sb = ots_tiles.pop(c)
                ts_ps = pool.tile([128, 4], F32, tag=ptag, name=f"ts{c}")
                for tb in range(4):
                    nc.tensor.transpose(
                        ts_ps[:, tb:tb + 1],
                        sums_sb[0:1, tb * 128:(tb + 1) * 128],
                        ident_sb[0:1, 0:1],
                    )
                recip_sb = small.tile([128, 4], F32, tag="recip")
                with nc.allow_low_precision(reason="fp32 reciprocal"):
                    nc.vector.reciprocal(recip_sb, ts_ps)
                for tb in range(4):
                    po = pool.tile([128, QC], F32, tag=ptag, name=f"po{c}_{tb}")
                    nc.tensor.matmul(
                        po,
                        ots_sb[:, tb * 128:(tb + 1) * 128],
                        wp_sb,
                        start=True,
                        stop=True,
                    )
                    o_sb = outp.tile([128, D], F32, tag="outs")
                    nc.vector.tensor_scalar_mul(o_sb, po, recip_sb[:, tb:tb + 1])
                    row = c * QC + tb * 128
                    nc.sync.dma_start(out=out_d[row:row + 128, :], in_=o_sb)

            # ---- flat software pipeline across all (chunk, group) steps ----
            # QK leads exp by one step; PV trails exp by one step; each
            # chunk's normalization + projection is emitted two steps after
            # its last group so it never blocks the next chunk's QK path.
            seq = [(c, g) for c in range(NQC) for g in range(ng)]
            prod_for_group(0, 0)
            prod_qt(0)
            for i, (c, g) in enumerate(seq):
                emit_qk(c, g)
                if i > 0:
                    pc, pg = seq[i - 1]
                    emit_pv(pc, pg)
                    if pg == ng - 1:
                        emit_copies(pc)
                emit_exp(c, g)
                prod_for_group(c, g + 1)
                prod_for_group(c, g + 2)
                if i > 1:
                    ppc, ppg = seq[i - 2]
                    if ppg == ng - 1:
                        emit_norm_po(ppc)
                if g == 4:
                    prod_qt(min(c + 1, NQC - 1))
            emit_pv(*seq[-1])
            emit_copies(NQC - 1)
            emit_norm_po(NQC - 1)

    nc.compile()
    return nc


def _prep_inputs(x, scale, Wq, Wkv, Wproj):
    """Per-core input maps (head h on core h)."""
    import ml_dtypes
    bf = ml_dtypes.bfloat16
    x2 = np.ascontiguousarray(x.reshape(N, D)).astype(bf)
    mask = (1.0 - np.eye(128)).astype(np.float32)
    identb = np.eye(128, dtype=np.float32)
    onesw = np.ones((128, HD), dtype=np.float32)
    ident = np.ones((1, 1), dtype=np.float32)
    in_maps = []
    for h in range(NH):
        wqh = Wq[h * HD:(h + 1) * HD, :]                  # [64, 512]
        wkh = Wkv[h * HD:(h + 1) * HD, :]
        wvh = Wkv[D + h * HD:D + (h + 1) * HD, :]
        # lhsT [c, m] with m duplicated halves -> [128, 4x128]
        def lhsT_dup(w):
            a = np.concatenate([w.T, w.T], axis=1)        # [512, 128]
            return np.ascontiguousarray(
                a.reshape(4, 128, 128).transpose(1, 0, 2))
        # V' rhs [c, 66] -> [128, 4, 66] (col 64 becomes the ones column)
        b = np.concatenate(
            [wvh.T, np.zeros((D, 2), dtype=np.float32)], axis=1)
        wv_host = np.ascontiguousarray(
            b.reshape(4, 128, HD + 2).transpose(1, 0, 2))
        wp_host = np.ascontiguousarray(
            Wproj[:, h * HD:(h + 1) * HD].T, dtype=np.float32)  # [64, 512]
        in_maps.append({
            "xinp": x2,
            "wq": np.ascontiguousarray(lhsT_dup(wqh)).astype(bf),
            "wk": np.ascontiguousarray(lhsT_dup(wkh)).astype(bf),
            "wv": np.ascontiguousarray(wv_host).astype(bf),
            "identb": identb.astype(bf),
            "mask": mask.astype(bf),
            "onesw": onesw.astype(bf),
            "wp": wp_host,
            "ident": ident,
        })
    return in_maps


def kernel(x, H, W, scale, Wq, Wkv, Wproj, bproj, _trace=False):
    global LAST_EXEC_TIME_NS
    x = np.asarray(x, dtype=np.float32)
    Wq = np.asarray(Wq, dtype=np.float32)
    Wkv = np.asarray(Wkv, dtype=np.float32)
    Wproj = np.asarray(Wproj, dtype=np.float32)
    bproj = np.asarray(bproj, dtype=np.float32)
    scale_val = float(np.asarray(scale).reshape(-1)[0])

    key = round(scale_val, 12)
    nc = _BUILD_CACHE.get(key)
    if nc is None:
        nc = _build(scale_val)
        _BUILD_CACHE[key] = nc

    in_maps = _prep_inputs(x, scale, Wq, Wkv, Wproj)
    try:
        res = run_bass_kernel_spmd(
            nc, in_maps, core_ids=list(range(NH)), trace=_trace)
    except Exception:
        # transient NRT device errors recover on retry
        res = run_bass_kernel_spmd(
            nc, in_maps, core_ids=list(range(NH)), trace=_trace)
    LAST_EXEC_TIME_NS = res.exec_time_ns

    acc = np.zeros((N, D), dtype=np.float64)
    for h in range(NH):
        acc += res.results[h]["out"]
    out = (acc + bproj.astype(np.float64)).astype(np.float32)
    return out.reshape(1, N, D)


# revision 43
# speedup vs baseline: 1.0336x; 1.0228x over previous
"""Multi-head attention (dense_transformer) on 8 TRN2 NeuronCores.

Reference computation (B=1, N=4096, D=512, 8 heads, head_dim 64):
    q = x @ Wq.T ; k, v = split(x @ Wkv.T)
    attn = softmax_masked(q k^T * scale)   # diagonal masked to zero
    out = (attn @ v) @ Wproj.T + bproj

Sharding: head-parallel — core h computes head h end to end, including its
partial output projection out_h = (attn_h @ v_h) @ Wproj[:, 64h:64h+64].T.
The host sums the 8 partials and adds the bias. No cross-core comm.

Per-core layout is fully "transposed" (channels on partitions) so that the
softmax normalizer comes out of the PV matmul via an appended ones column:
    XT  [c=512, n=4096]  (TensorE transposes of x tiles)
    QT/KT [128, 4096]    rows 0..63 = head projection, rows 64..127 = copy
                         (duplicate halves enable 2-way row-packed QK matmuls)
    ST strip j = K_j @ Q^T -> [128 keys, 512 queries] in PSUM
    E = exp(scale * ST)  (ScalarE, no max subtraction needed: |scores| < ~2)
    diagonal blocks of E multiplied by (1 - I) to zero self-attention
    OT'[65, 512] += V'_j^T @ E_j  over key strips; row 64 = sum of exps
    PO[tok, 512] = OT'[0:64, tok-block].T @ WprojSlice^T
    out[tok, :]  = PO * (1/sums)[tok]   (per-partition tensor_scalar)

The preamble (XT transposes, KT/V' strips, QT chunks) is produced
just-in-time inside chunk 0 so the ScalarEngine starts exp'ing early.
"""

import numpy as np

import concourse.bass as bass
import concourse.tile as tile
from concourse import bacc, mybir
from concourse.bass_utils import run_bass_kernel_spmd

F32 = mybir.dt.float32
F32R = mybir.dt.float32r
BF16 = mybir.dt.bfloat16
EXP = mybir.ActivationFunctionType.Exp

N = 4096
D = 512
NH = 8
HD = 64
NQC = 8          # query chunks of 512
QC = 512
NST = 32         # key strips of 128
GS = 3           # key strips per exp group

LAST_EXEC_TIME_NS = None
_BUILD_CACHE = {}


def _groups():
    """Key-strip groups per query chunk: [(start, size), ...]."""
    out = []
    j = 0
    while j < NST:
        out.append((j, min(GS, NST - j)))
        j += GS
    return out


def _build(scale_val: float):
    nc = bacc.Bacc("TRN2", target_bir_lowering=False, debug=False)

    x_d = nc.dram_tensor("xinp", [N, D], BF16, kind="ExternalInput").ap()
    wq_d = nc.dram_tensor("wq", [128, 4, 128], BF16, kind="ExternalInput").ap()
    wk_d = nc.dram_tensor("wk", [128, 4, 128], BF16, kind="ExternalInput").ap()
    wv_d = nc.dram_tensor("wv", [128, 4, HD + 2], BF16, kind="ExternalInput").ap()
    identb_d = nc.dram_tensor("identb", [128, 128], BF16, kind="ExternalInput").ap()
    mask_d = nc.dram_tensor("mask", [128, 128], BF16, kind="ExternalInput").ap()
    ones_d = nc.dram_tensor("onesw", [128, HD], BF16, kind="ExternalInput").ap()
    wp_d = nc.dram_tensor("wp", [HD, D], F32, kind="ExternalInput").ap()
    ident_d = nc.dram_tensor("ident", [1, 1], F32, kind="ExternalInput").ap()
    out_d = nc.dram_tensor("out", [N, D], F32, kind="ExternalOutput").ap()

    groups = _groups()
    ng = len(groups)

    with tile.TileContext(nc) as tc:
        with (
            tc.tile_pool(name="consts", bufs=1) as consts,
            tc.tile_pool(name="persist", bufs=1) as persist,
            tc.tile_pool(name="xin", bufs=8) as xin,
            tc.tile_pool(name="epool", bufs=6) as epool,
            tc.tile_pool(name="small", bufs=2) as small,
            tc.tile_pool(name="outp", bufs=4) as outp,
            tc.tile_pool(name="ps_st", bufs=2, space="PSUM") as ps_st,
            tc.tile_pool(name="ps_ot", bufs=1, space="PSUM") as ps_ot,
            tc.tile_pool(name="ps_misc", bufs=1, space="PSUM") as ps_misc,
        ):
            # ---- constants ----
            wq_sb = consts.tile([128, 4, 128], BF16, tag="wq")
            wk_sb = consts.tile([128, 4, 128], BF16, tag="wk")
            wv_sb = consts.tile([128, 4, HD + 2], BF16, tag="wv")
            identb_sb = consts.tile([128, 128], BF16, tag="identb")
            mask_sb = consts.tile([128, 128], BF16, tag="mask")
            ones_sb = consts.tile([128, HD], BF16, tag="ones")
            wp_f32 = consts.tile([HD, D], F32, tag="wpf")
            wp_sb = consts.tile([HD, D], F32R, tag="wp")
            ident_sb = consts.tile([1, 1], F32, tag="ident")
            nc.sync.dma_start(out=identb_sb, in_=identb_d)
            x_pre = []
            for t in range(8):
                x_t = xin.tile([128, D], BF16, tag="xin", name=f"x{t}")
                nc.sync.dma_start(out=x_t, in_=x_d[t * 128:(t + 1) * 128, :])
                x_pre.append(x_t)
            nc.sync.dma_start(out=wq_sb, in_=wq_d)
            nc.sync.dma_start(out=wk_sb, in_=wk_d)
            nc.sync.dma_start(out=wv_sb, in_=wv_d)
            nc.sync.dma_start(out=mask_sb, in_=mask_d)
            nc.sync.dma_start(out=ones_sb, in_=ones_d)
            nc.sync.dma_start(out=wp_f32, in_=wp_d)
            nc.vector.tensor_copy(wp_sb, wp_f32)
            nc.sync.dma_start(out=ident_sb, in_=ident_d)

            # preload the exp activation table while DMAs stream
            scratch = consts.tile([1, 2], F32, tag="scratch")
            nc.vector.memset(scratch, 0.0)
            nc.scalar.activation(scratch, scratch, EXP)

            # ---- persistent tensors ----
            xt = persist.tile([128, 4, N], BF16, tag="xt")     # XT[c%128, c//128, n]
            qt = persist.tile([128, N], BF16, tag="qt")        # QT duplicated halves
            kt = persist.tile([128, N], BF16, tag="kt")
            vp = persist.tile([128, NST, HD + 2], BF16, tag="vp")

            # ---- preamble: XT via TensorE transposes, produced lazily ----
            t_done = [False] * NST
            kt_done = [False] * NQC
            v_done = [False] * NST
            qt_done = [False] * NQC

            # tokens 0..1023 go through the TensorEngine (their DMAs are
            # issued first so the xbar bulk transfer doesn't delay them);
            # tokens 1024..4095 arrive via xbar DMA transpose behind them.
            for b0 in range(1024, N, 1024):
                for cb in range(4):
                    nc.sync.dma_start_transpose(
                        out=xt[:, cb, b0:b0 + 1024],
                        in_=x_d[b0:b0 + 1024, cb * 128:(cb + 1) * 128],
                    )
            for t_ in range(8, NST):
                t_done[t_] = True

            def prod_t(t):
                if t_done[t]:
                    return
                t_done[t] = True
                x_t = x_pre[t]
                tr = ps_st.tile([128, D], BF16, tag="st", name=f"tr{t}")
                for cb in range(4):
                    nc.tensor.transpose(
                        tr[:, cb * 128:(cb + 1) * 128],
                        x_t[:, cb * 128:(cb + 1) * 128],
                        identb_sb,
                    )
                nc.vector.tensor_copy(
                    xt[:, 0:4, t * 128:(t + 1) * 128],
                    tr.rearrange("p (cb tt) -> p cb tt", cb=4),
                )

            def prod_xt(qc):
                for t in range(4 * qc, 4 * qc + 4):
                    prod_t(t)

            def prod_kt(qc):
                if kt_done[qc]:
                    return
                kt_done[qc] = True
                prod_xt(qc)
                sl = slice(qc * QC, (qc + 1) * QC)
                pp = ps_st.tile([128, QC], F32, tag="st")
                for cc in range(4):
                    nc.tensor.matmul(
                        pp, wk_sb[:, cc, :], xt[:, cc, sl],
                        start=(cc == 0), stop=(cc == 3),
                    )
                nc.vector.tensor_copy(kt[:, sl], pp)

            def prod_qt(qc):
                if qt_done[qc]:
                    return
                qt_done[qc] = True
                prod_xt(qc)
                sl = slice(qc * QC, (qc + 1) * QC)
                pp = ps_misc.tile([128, QC], F32, tag="misc", name=f"qtp{qc}")
                for cc in range(4):
                    nc.tensor.matmul(
                        pp, wq_sb[:, cc, :], xt[:, cc, sl],
                        start=(cc == 0), stop=(cc == 3),
                    )
                nc.vector.tensor_copy(qt[:, sl], pp)

            def prod_v(t):
                if v_done[t]:
                    return
                v_done[t] = True
                prod_xt(t // 4)
                vv = ps_misc.tile([128, HD + 2], F32, tag="misc", name=f"vv{t}")
                for cc in range(4):
                    nc.tensor.matmul(
                        vv, xt[:, cc, t * 128:(t + 1) * 128], wv_sb[:, cc, :],
                        start=(cc == 0), stop=(cc == 3),
                    )
                nc.vector.tensor_copy(vp[:, t, :], vv)
                nc.vector.tensor_copy(vp[:, t, HD:HD + 1], ones_sb[:, 0:1])

            def prod_for_group(c, g):
                if c > 0 or g >= ng:
                    return
                j0, gsz = groups[g]
                for i in range(gsz):
                    j = j0 + i
                    prod_kt(j // 4)
                    prod_v(j)

            # ---- per-chunk state ----
            ot_tiles = {}
            e_tiles = {}
            st_tiles = {}
            sums_tiles = {}
            ots_tiles = {}

            def emit_qk(c, g):
                j0, gsz = groups[g]
                qsl = slice(c * QC, (c + 1) * QC)
                st = ps_st.tile([128, GS, QC], F32, tag="st")
                st_tiles[(c, g)] = st
                for i in range(gsz):
                    j = j0 + i
                    ro = 64 * (j % 2)
                    nc.tensor.matmul(
                        st[:, i, :],
                        kt[ro:ro + 64, j * 128:(j + 1) * 128],
                        qt[ro:ro + 64, qsl],
                        start=True,
                        stop=True,
                    )

            def emit_exp(c, g):
                j0, gsz = groups[g]
                e_t = epool.tile([128, GS, QC], BF16, tag="e")
                e_tiles[(c, g)] = e_t
                nc.scalar.activation(
                    e_t[:, 0:gsz, :],
                    st_tiles.pop((c, g))[:, 0:gsz, :],
                    EXP,
                    scale=scale_val,
                )
                for i in range(gsz):
                    j = j0 + i
                    r = j - 4 * c
                    if 0 <= r < 4:
                        blk = e_t[:, i, r * 128:(r + 1) * 128]
                        nc.vector.tensor_mul(blk, blk, mask_sb)

            def emit_pv(c, g):
                j0, gsz = groups[g]
                if g == 0:
                    ot_tiles[c] = ps_ot.tile([128, QC], F32, tag="ot", name=f"ot{c}")
                ot = ot_tiles[c]
                e_t = e_tiles.pop((c, g))
                for i in range(gsz):
                    j = j0 + i
                    nc.tensor.matmul(
                        ot[0:HD + 1, :],
                        vp[:, j, 0:HD + 1],
                        e_t[:, i, :],
                        start=(j == 0),
                        stop=(j == NST - 1),
                        skip_group_check=True,
                    )

            def emit_copies(c):
                # pull sums row + unnormalized OT out of PSUM; frees the ot bank
                ot = ot_tiles.pop(c)
                sums_sb = small.tile([1, QC], F32, tag="sums")
                nc.vector.tensor_copy(sums_sb, ot[HD:HD + 1, :])
                ots_sb = small.tile([HD, QC], F32R, tag="ots")
                nc.vector.tensor_copy(ots_sb, ot[0:HD, :])
                sums_tiles[c] = sums_sb
                ots_tiles[c] = ots_sb

            def emit_norm_po(c):
                pool, ptag = ((ps_st, "st") if c == NQC - 1
                              else (ps_misc, "misc"))
                sums_sb = sums_tiles.pop(c)
                ots_sb = ots_tiles.pop(c)
                ts_ps = pool.tile([128, 4], F32, tag=ptag, name=f"ts{c}")
                for tb in range(4):
                    nc.tensor.transpose(
                        ts_ps[:, tb:tb + 1],
                        sums_sb[0:1, tb * 128:(tb + 1) * 128],
                        ident_sb[0:1, 0:1],
                    )
                recip_sb = small.tile([128, 4], F32, tag="recip")
                with nc.allow_low_precision(reason="fp32 reciprocal"):
                    nc.vector.reciprocal(recip_sb, ts_ps)
                for tb in range(4):
                    po = pool.tile([128, QC], F32, tag=ptag, name=f"po{c}_{tb}")
                    nc.tensor.matmul(
                        po,
                        ots_sb[:, tb * 128:(tb + 1) * 128],
                        wp_sb,
                        start=True,
                        stop=True,
                    )
                    o_sb = outp.tile([128, D], F32, tag="outs")
                    nc.vector.tensor_scalar_mul(o_sb, po, recip_sb[:, tb:tb + 1])
                    row = c * QC + tb * 128
                    nc.sync.dma_start(out=out_d[row:row + 128, :], in_=o_sb)

            # ---- flat software pipeline across all (chunk, group) steps ----
            # QK leads exp by one step; PV trails exp by one step; each
            # chunk's normalization + projection is emitted two steps after
            # its last group so it never blocks the next chunk's QK path.
            seq = [(c, g) for c in range(NQC) for g in range(ng)]
            prod_for_group(0, 0)
            prod_qt(0)
            for i, (c, g) in enumerate(seq):
                emit_qk(c, g)
                if i > 0:
                    pc, pg = seq[i - 1]
                    emit_pv(pc, pg)
                    if pg == ng - 1:
                        emit_copies(pc)
                emit_exp(c, g)
                prod_for_group(c, g + 1)
                prod_for_group(c, g + 2)
                if i > 1:
                    ppc, ppg = seq[i - 2]
                    if ppg == ng - 1:
                        emit_norm_po(ppc)
                if g == 4:
                    prod_qt(min(c + 1, NQC - 1))
            emit_pv(*seq[-1])
            emit_copies(NQC - 1)
            emit_norm_po(NQC - 1)

    nc.compile()
    return nc


def _prep_inputs(x, scale, Wq, Wkv, Wproj):
    """Per-core input maps (head h on core h)."""
    import ml_dtypes
    bf = ml_dtypes.bfloat16
    x2 = np.ascontiguousarray(x.reshape(N, D)).astype(bf)
    mask = (1.0 - np.eye(128)).astype(np.float32)
    identb = np.eye(128, dtype=np.float32)
    onesw = np.ones((128, HD), dtype=np.float32)
    ident = np.ones((1, 1), dtype=np.float32)
    in_maps = []
    for h in range(NH):
        wqh = Wq[h * HD:(h + 1) * HD, :]                  # [64, 512]
        wkh = Wkv[h * HD:(h + 1) * HD, :]
        wvh = Wkv[D + h * HD:D + (h + 1) * HD, :]
        # lhsT [c, m] with m duplicated halves -> [128, 4x128]
        def lhsT_dup(w):
            a = np.concatenate([w.T, w.T], axis=1)        # [512, 128]
            return np.ascontiguousarray(
                a.reshape(4, 128, 128).transpose(1, 0, 2))
        # V' rhs [c, 66] -> [128, 4, 66] (col 64 becomes the ones column)
        b = np.concatenate(
            [wvh.T, np.zeros((D, 2), dtype=np.float32)], axis=1)
        wv_host = np.ascontiguousarray(
            b.reshape(4, 128, HD + 2).transpose(1, 0, 2))
        wp_host = np.ascontiguousarray(
            Wproj[:, h * HD:(h + 1) * HD].T, dtype=np.float32)  # [64, 512]
        in_maps.append({
            "xinp": x2,
            "wq": np.ascontiguousarray(lhsT_dup(wqh)).astype(bf),
            "wk": np.ascontiguousarray(lhsT_dup(wkh)).astype(bf),
            "wv": np.ascontiguousarray(wv_host).astype(bf),
            "identb": identb.astype(bf),
            "mask": mask.astype(bf),
            "onesw": onesw.astype(bf),
            "wp": wp_host,
            "ident": ident,
        })
    return in_maps


def kernel(x, H, W, scale, Wq, Wkv, Wproj, bproj, _trace=False):
    global LAST_EXEC_TIME_NS
    x = np.asarray(x, dtype=np.float32)
    Wq = np.asarray(Wq, dtype=np.float32)
    Wkv = np.asarray(Wkv, dtype=np.float32)
    Wproj = np.asarray(Wproj, dtype=np.float32)
    bproj = np.asarray(bproj, dtype=np.float32)
    scale_val = float(np.asarray(scale).reshape(-1)[0])

    key = round(scale_val, 12)
    nc = _BUILD_CACHE.get(key)
    if nc is None:
        nc = _build(scale_val)
        _BUILD_CACHE[key] = nc

    in_maps = _prep_inputs(x, scale, Wq, Wkv, Wproj)
    try:
        res = run_bass_kernel_spmd(
            nc, in_maps, core_ids=list(range(NH)), trace=_trace)
    except Exception:
        # transient NRT device errors recover on retry
        res = run_bass_kernel_spmd(
            nc, in_maps, core_ids=list(range(NH)), trace=_trace)
    LAST_EXEC_TIME_NS = res.exec_time_ns

    acc = np.zeros((N, D), dtype=np.float64)
    for h in range(NH):
        acc += res.results[h]["out"]
    out = (acc + bproj.astype(np.float64)).astype(np.float32)
    return out.reshape(1, N, D)


# revision 44
# speedup vs baseline: 1.0505x; 1.0164x over previous
"""Multi-head attention (dense_transformer) on 8 TRN2 NeuronCores.

Reference computation (B=1, N=4096, D=512, 8 heads, head_dim 64):
    q = x @ Wq.T ; k, v = split(x @ Wkv.T)
    attn = softmax_masked(q k^T * scale)   # diagonal masked to zero
    out = (attn @ v) @ Wproj.T + bproj

Sharding: head-parallel — core h computes head h end to end, including its
partial output projection out_h = (attn_h @ v_h) @ Wproj[:, 64h:64h+64].T.
The host sums the 8 partials and adds the bias. No cross-core comm.

Per-core layout is fully "transposed" (channels on partitions) so that the
softmax normalizer comes out of the PV matmul via an appended ones column:
    XT  [c=512, n=4096]  (TensorE transposes of x tiles)
    QT/KT [128, 4096]    rows 0..63 = head projection, rows 64..127 = copy
                         (duplicate halves enable 2-way row-packed QK matmuls)
    ST strip j = K_j @ Q^T -> [128 keys, 512 queries] in PSUM
    E = exp(scale * ST)  (ScalarE, no max subtraction needed: |scores| < ~2)
    diagonal blocks of E multiplied by (1 - I) to zero self-attention
    OT'[65, 512] += V'_j^T @ E_j  over key strips; row 64 = sum of exps
    PO[tok, 512] = OT'[0:64, tok-block].T @ WprojSlice^T
    out[tok, :]  = PO * (1/sums)[tok]   (per-partition tensor_scalar)

The preamble (XT transposes, KT/V' strips, QT chunks) is produced
just-in-time inside chunk 0 so the ScalarEngine starts exp'ing early.
"""

import numpy as np

import concourse.bass as bass
import concourse.tile as tile
from concourse import bacc, mybir
from concourse.bass_utils import run_bass_kernel_spmd

F32 = mybir.dt.float32
F32R = mybir.dt.float32r
BF16 = mybir.dt.bfloat16
EXP = mybir.ActivationFunctionType.Exp

N = 4096
D = 512
NH = 8
HD = 64
NQC = 8          # query chunks of 512
QC = 512
NST = 32         # key strips of 128
GS = 3           # key strips per exp group

LAST_EXEC_TIME_NS = None
_BUILD_CACHE = {}


def _groups():
    """Key-strip groups per query chunk: [(start, size), ...]."""
    out = []
    j = 0
    while j < NST:
        out.append((j, min(GS, NST - j)))
        j += GS
    return out


def _build(scale_val: float):
    nc = bacc.Bacc("TRN2", target_bir_lowering=False, debug=False)

    x_d = nc.dram_tensor("xinp", [N, D], BF16, kind="ExternalInput").ap()
    wq_d = nc.dram_tensor("wq", [128, 4, 128], BF16, kind="ExternalInput").ap()
    wk_d = nc.dram_tensor("wk", [128, 4, 128], BF16, kind="ExternalInput").ap()
    wv_d = nc.dram_tensor("wv", [128, 4, HD + 2], BF16, kind="ExternalInput").ap()
    identb_d = nc.dram_tensor("identb", [128, 128], BF16, kind="ExternalInput").ap()
    mask_d = nc.dram_tensor("mask", [128, 128], BF16, kind="ExternalInput").ap()
    ones_d = nc.dram_tensor("onesw", [128, HD], BF16, kind="ExternalInput").ap()
    wp_d = nc.dram_tensor("wp", [HD, D], F32, kind="ExternalInput").ap()
    ident_d = nc.dram_tensor("ident", [1, 1], F32, kind="ExternalInput").ap()
    out_d = nc.dram_tensor("out", [N, D], F32, kind="ExternalOutput").ap()

    groups = _groups()
    ng = len(groups)

    with tile.TileContext(nc) as tc:
        with (
            tc.tile_pool(name="consts", bufs=1) as consts,
            tc.tile_pool(name="persist", bufs=1) as persist,
            tc.tile_pool(name="xin", bufs=8) as xin,
            tc.tile_pool(name="epool", bufs=6) as epool,
            tc.tile_pool(name="small", bufs=2) as small,
            tc.tile_pool(name="outp", bufs=4) as outp,
            tc.tile_pool(name="ps_st", bufs=2, space="PSUM") as ps_st,
            tc.tile_pool(name="ps_ot", bufs=1, space="PSUM") as ps_ot,
            tc.tile_pool(name="ps_misc", bufs=1, space="PSUM") as ps_misc,
        ):
            # ---- constants ----
            wq_sb = consts.tile([128, 4, 128], BF16, tag="wq")
            wk_sb = consts.tile([128, 4, 128], BF16, tag="wk")
            wv_sb = consts.tile([128, 4, HD + 2], BF16, tag="wv")
            identb_sb = consts.tile([128, 128], BF16, tag="identb")
            mask_sb = consts.tile([128, 128], BF16, tag="mask")
            ones_sb = consts.tile([128, HD], BF16, tag="ones")
            wp_f32 = consts.tile([HD, D], F32, tag="wpf")
            wp_sb = consts.tile([HD, D], F32R, tag="wp")
            ident_sb = consts.tile([1, 1], F32, tag="ident")
            nc.sync.dma_start(out=identb_sb, in_=identb_d)
            x_pre = []
            for t in range(8):
                x_t = xin.tile([128, D], BF16, tag="xin", name=f"x{t}")
                nc.sync.dma_start(out=x_t, in_=x_d[t * 128:(t + 1) * 128, :])
                x_pre.append(x_t)
            nc.sync.dma_start(out=wq_sb, in_=wq_d)
            nc.sync.dma_start(out=wk_sb, in_=wk_d)
            nc.sync.dma_start(out=wv_sb, in_=wv_d)
            nc.sync.dma_start(out=mask_sb, in_=mask_d)
            nc.sync.dma_start(out=ones_sb, in_=ones_d)
            nc.sync.dma_start(out=wp_f32, in_=wp_d)
            nc.vector.tensor_copy(wp_sb, wp_f32)
            nc.sync.dma_start(out=ident_sb, in_=ident_d)

            # preload the exp activation table while DMAs stream
            scratch = consts.tile([1, 2], F32, tag="scratch")
            nc.vector.memset(scratch, 0.0)
            nc.scalar.activation(scratch, scratch, EXP)

            # ---- persistent tensors ----
            xt = persist.tile([128, 4, N], BF16, tag="xt")     # XT[c%128, c//128, n]
            qt = persist.tile([128, N], BF16, tag="qt")        # QT duplicated halves
            kt = persist.tile([128, N], BF16, tag="kt")
            vp = persist.tile([128, NST, HD + 2], BF16, tag="vp")

            # ---- preamble: XT via TensorE transposes, produced lazily ----
            t_done = [False] * NST
            kt_done = [False] * NQC
            v_done = [False] * NST
            qt_done = [False] * NQC

            # tokens 0..1023 go through the TensorEngine (their DMAs are
            # issued first so the xbar bulk transfer doesn't delay them);
            # tokens 1024..4095 arrive via xbar DMA transpose behind them.
            for b0 in range(1024, N, 1024):
                for cb in range(4):
                    nc.sync.dma_start_transpose(
                        out=xt[:, cb, b0:b0 + 1024],
                        in_=x_d[b0:b0 + 1024, cb * 128:(cb + 1) * 128],
                    )
            for t_ in range(8, NST):
                t_done[t_] = True

            def prod_t(t):
                if t_done[t]:
                    return
                t_done[t] = True
                x_t = x_pre[t]
                tr = ps_st.tile([128, D], BF16, tag="st", name=f"tr{t}")
                for cb in range(4):
                    nc.tensor.transpose(
                        tr[:, cb * 128:(cb + 1) * 128],
                        x_t[:, cb * 128:(cb + 1) * 128],
                        identb_sb,
                    )
                nc.vector.tensor_copy(
                    xt[:, 0:4, t * 128:(t + 1) * 128],
                    tr.rearrange("p (cb tt) -> p cb tt", cb=4),
                )

            def prod_xt(qc):
                for t in range(4 * qc, 4 * qc + 4):
                    prod_t(t)

            def prod_kt(qc):
                if kt_done[qc]:
                    return
                kt_done[qc] = True
                prod_xt(qc)
                sl = slice(qc * QC, (qc + 1) * QC)
                pp = ps_st.tile([128, QC], F32, tag="st")
                for cc in range(4):
                    nc.tensor.matmul(
                        pp, wk_sb[:, cc, :], xt[:, cc, sl],
                        start=(cc == 0), stop=(cc == 3),
                    )
                nc.vector.tensor_copy(kt[:, sl], pp)

            def prod_qt(qc):
                if qt_done[qc]:
                    return
                qt_done[qc] = True
                prod_xt(qc)
                sl = slice(qc * QC, (qc + 1) * QC)
                pp = ps_misc.tile([128, QC], F32, tag="misc", name=f"qtp{qc}")
                for cc in range(4):
                    nc.tensor.matmul(
                        pp, wq_sb[:, cc, :], xt[:, cc, sl],
                        start=(cc == 0), stop=(cc == 3),
                    )
                nc.vector.tensor_copy(qt[:, sl], pp)

            def prod_v(t):
                if v_done[t]:
                    return
                v_done[t] = True
                prod_xt(t // 4)
                vv = ps_misc.tile([128, HD + 2], F32, tag="misc", name=f"vv{t}")
                for cc in range(4):
                    nc.tensor.matmul(
                        vv, xt[:, cc, t * 128:(t + 1) * 128], wv_sb[:, cc, :],
                        start=(cc == 0), stop=(cc == 3),
                    )
                nc.vector.tensor_copy(vp[:, t, :], vv)
                nc.vector.tensor_copy(vp[:, t, HD:HD + 1], ones_sb[:, 0:1])

            def prod_for_group(c, g):
                if c > 0 or g >= ng:
                    return
                j0, gsz = groups[g]
                for i in range(gsz):
                    j = j0 + i
                    prod_kt(j // 4)
                    prod_v(j)

            # ---- per-chunk state ----
            ot_tiles = {}
            e_tiles = {}
            st_tiles = {}
            sums_tiles = {}
            ots_tiles = {}

            def emit_qk(c, g):
                j0, gsz = groups[g]
                qsl = slice(c * QC, (c + 1) * QC)
                st = ps_st.tile([128, GS, QC], F32, tag="st")
                st_tiles[(c, g)] = st
                for i in range(gsz):
                    j = j0 + i
                    ro = 64 * (j % 2)
                    nc.tensor.matmul(
                        st[:, i, :],
                        kt[ro:ro + 64, j * 128:(j + 1) * 128],
                        qt[ro:ro + 64, qsl],
                        start=True,
                        stop=True,
                    )

            def emit_exp(c, g):
                j0, gsz = groups[g]
                e_t = epool.tile([128, GS, QC], BF16, tag="e")
                e_tiles[(c, g)] = e_t
                nc.scalar.activation(
                    e_t[:, 0:gsz, :],
                    st_tiles.pop((c, g))[:, 0:gsz, :],
                    EXP,
                    scale=scale_val,
                )
                for i in range(gsz):
                    j = j0 + i
                    r = j - 4 * c
                    if 0 <= r < 4:
                        blk = e_t[:, i, r * 128:(r + 1) * 128]
                        nc.vector.tensor_mul(blk, blk, mask_sb)

            def emit_pv(c, g):
                j0, gsz = groups[g]
                if g == 0:
                    ot_tiles[c] = ps_ot.tile([128, QC], F32, tag="ot", name=f"ot{c}")
                ot = ot_tiles[c]
                e_t = e_tiles.pop((c, g))
                for i in range(gsz):
                    j = j0 + i
                    nc.tensor.matmul(
                        ot[0:HD + 1, :],
                        vp[:, j, 0:HD + 1],
                        e_t[:, i, :],
                        start=(j == 0),
                        stop=(j == NST - 1),
                        skip_group_check=True,
                    )

            def emit_copies(c):
                # pull sums row + unnormalized OT out of PSUM; frees the ot bank
                ot = ot_tiles.pop(c)
                sums_sb = small.tile([1, QC], F32, tag="sums")
                nc.vector.tensor_copy(sums_sb, ot[HD:HD + 1, :])
                ots_sb = small.tile([HD, QC], F32R, tag="ots")
                nc.vector.tensor_copy(ots_sb, ot[0:HD, :])
                sums_tiles[c] = sums_sb
                ots_tiles[c] = ots_sb

            def emit_norm_po(c):
                pool, ptag = ((ps_st, "st") if c == NQC - 1
                              else (ps_misc, "misc"))
                sums_sb = sums_tiles.pop(c)
                ots_sb = ots_tiles.pop(c)
                ts_ps = pool.tile([128, 4], F32, tag=ptag, name=f"ts{c}")
                for tb in range(4):
                    nc.tensor.transpose(
                        ts_ps[:, tb:tb + 1],
                        sums_sb[0:1, tb * 128:(tb + 1) * 128],
                        ident_sb[0:1, 0:1],
                    )
                recip_sb = small.tile([128, 4], F32, tag="recip")
                with nc.allow_low_precision(reason="fp32 reciprocal"):
                    nc.vector.reciprocal(recip_sb, ts_ps)
                for tb in range(4):
                    po = pool.tile([128, QC], F32, tag=ptag, name=f"po{c}_{tb}")
                    nc.tensor.matmul(
                        po,
                        ots_sb[:, tb * 128:(tb + 1) * 128],
                        wp_sb,
                        start=True,
                        stop=True,
                    )
                    o_sb = outp.tile([128, D], F32, tag="outs")
                    nc.vector.tensor_scalar_mul(o_sb, po, recip_sb[:, tb:tb + 1])
                    row = c * QC + tb * 128
                    nc.sync.dma_start(out=out_d[row:row + 128, :], in_=o_sb)

            # ---- flat software pipeline across all (chunk, group) steps ----
            # QK leads exp by one step; PV trails exp by one step; each
            # chunk's normalization + projection is emitted two steps after
            # its last group so it never blocks the next chunk's QK path.
            seq = [(c, g) for c in range(NQC) for g in range(ng)]
            prod_qt(0)
            prod_for_group(0, 0)
            for i, (c, g) in enumerate(seq):
                emit_qk(c, g)
                if i > 0:
                    pc, pg = seq[i - 1]
                    emit_pv(pc, pg)
                    if pg == ng - 1:
                        emit_copies(pc)
                emit_exp(c, g)
                prod_for_group(c, g + 1)
                prod_for_group(c, g + 2)
                if i > 1:
                    ppc, ppg = seq[i - 2]
                    if ppg == ng - 1:
                        emit_norm_po(ppc)
                if g == 4:
                    prod_qt(min(c + 1, NQC - 1))
            emit_pv(*seq[-1])
            emit_copies(NQC - 1)
            emit_norm_po(NQC - 1)

    nc.compile()
    return nc


def _prep_inputs(x, scale, Wq, Wkv, Wproj):
    """Per-core input maps (head h on core h)."""
    import ml_dtypes
    bf = ml_dtypes.bfloat16
    x2 = np.ascontiguousarray(x.reshape(N, D)).astype(bf)
    mask = (1.0 - np.eye(128)).astype(np.float32)
    identb = np.eye(128, dtype=np.float32)
    onesw = np.ones((128, HD), dtype=np.float32)
    ident = np.ones((1, 1), dtype=np.float32)
    in_maps = []
    for h in range(NH):
        wqh = Wq[h * HD:(h + 1) * HD, :]                  # [64, 512]
        wkh = Wkv[h * HD:(h + 1) * HD, :]
        wvh = Wkv[D + h * HD:D + (h + 1) * HD, :]
        # lhsT [c, m] with m duplicated halves -> [128, 4x128]
        def lhsT_dup(w):
            a = np.concatenate([w.T, w.T], axis=1)        # [512, 128]
            return np.ascontiguousarray(
                a.reshape(4, 128, 128).transpose(1, 0, 2))
        # V' rhs [c, 66] -> [128, 4, 66] (col 64 becomes the ones column)
        b = np.concatenate(
            [wvh.T, np.zeros((D, 2), dtype=np.float32)], axis=1)
        wv_host = np.ascontiguousarray(
            b.reshape(4, 128, HD + 2).transpose(1, 0, 2))
        wp_host = np.ascontiguousarray(
            Wproj[:, h * HD:(h + 1) * HD].T, dtype=np.float32)  # [64, 512]
        in_maps.append({
            "xinp": x2,
            "wq": np.ascontiguousarray(lhsT_dup(wqh)).astype(bf),
            "wk": np.ascontiguousarray(lhsT_dup(wkh)).astype(bf),
            "wv": np.ascontiguousarray(wv_host).astype(bf),
            "identb": identb.astype(bf),
            "mask": mask.astype(bf),
            "onesw": onesw.astype(bf),
            "wp": wp_host,
            "ident": ident,
        })
    return in_maps


def kernel(x, H, W, scale, Wq, Wkv, Wproj, bproj, _trace=False):
    global LAST_EXEC_TIME_NS
    x = np.asarray(x, dtype=np.float32)
    Wq = np.asarray(Wq, dtype=np.float32)
    Wkv = np.asarray(Wkv, dtype=np.float32)
    Wproj = np.asarray(Wproj, dtype=np.float32)
    bproj = np.asarray(bproj, dtype=np.float32)
    scale_val = float(np.asarray(scale).reshape(-1)[0])

    key = round(scale_val, 12)
    nc = _BUILD_CACHE.get(key)
    if nc is None:
        nc = _build(scale_val)
        _BUILD_CACHE[key] = nc

    in_maps = _prep_inputs(x, scale, Wq, Wkv, Wproj)
    try:
        res = run_bass_kernel_spmd(
            nc, in_maps, core_ids=list(range(NH)), trace=_trace)
    except Exception:
        # transient NRT device errors recover on retry
        res = run_bass_kernel_spmd(
            nc, in_maps, core_ids=list(range(NH)), trace=_trace)
    LAST_EXEC_TIME_NS = res.exec_time_ns

    acc = np.zeros((N, D), dtype=np.float64)
    for h in range(NH):
        acc += res.results[h]["out"]
    out = (acc + bproj.astype(np.float64)).astype(np.float32)
    return out.reshape(1, N, D)
